# revision 6
# baseline (speedup 1.0000x reference)
"""RBF/ARD covariance kernel K = exp(2*sn - 0.5 * ||s*(u_i - v_j)||^2) on 8 trn2 cores.

Strategy (sharding_hint): shard U rows across the 8 cores (each computes a
[1024, 8192] strip of K); V / weights / sn replicated.

Math: K = exp(E), E = 2*sn - 0.5*u2_i - 0.5*v2_j + (Us @ Vs.T)_ij with
Us = U*s, Vs = V*s, s = exp(-weights[:,0]), u2/v2 squared row norms of the
QUANTIZED Us/Vs (so E <= 2*sn up to fp32 accumulation noise and the
reference's max(sq,0) clamp cannot produce a visible difference).

Per core: fp8e4 GEMM with DoubleRow (contraction 512 = 2 passes of 2x128)
accumulated in fp32 PSUM; DVE adds the -0.5*v2_j broadcast row; ACT applies
exp(x + (2*sn - 0.5*u2_i)) via per-partition bias; bf16 store, host casts to
fp32. Falls back to bf16 GEMM if the scaled inputs exceed fp8e4 range.

Scheduling notes (from trace analysis):
- each dma_start costs ~650ns DGE issue time on its queue, and small strided
  descriptors kill early DMA bandwidth, so DRAM layouts are chunked to be
  per-partition contiguous per load ([P, chunk, k, 2, 512] for V);
- critical-path inputs are split over both HWDGE rings (sync + scalar);
- output DMAs go on the (otherwise idle) GpSimd SWDGE queue;
- DVE is the steady-state pacer (2222ns per [128,2048] group); ramp and
  tail use narrow chains so the elementwise pipeline starts/ends fast.
"""

import numpy as np
import ml_dtypes

N, M, D = 8192, 8192, 512
NCORES = 8
NLOC = N // NCORES          # 1024 U-rows per core
P = 128                     # partitions
KT = D // P                 # 4 contraction tiles of 128
KP = KT // 2                # 2 DoubleRow passes (2 k-tiles each)
IT = NLOC // P              # 8 i-tiles per core
JBLK = 512                  # matmul free dim (one PSUM bank fp32)
JG = 2048                   # j-group width (4 banks) for DVE/ACT/DMA batching
NJG = M // JG               # 4 j-groups
NJB = JG // JBLK            # 4 matmul j-blocks per group
NCH = M // JBLK             # 16 vst chunks of 512

F8 = ml_dtypes.float8_e4m3  # TRN float8e4 (max normal 240)
BF16 = ml_dtypes.bfloat16
FP8_MAX = 200.0             # safety margin under 240

_cache = {}


def _build(use_fp8, out_fp8):
    import concourse.bass as bass
    import concourse.mybir as mybir
    import concourse.tile as tile
    from concourse import bacc

    F32 = mybir.dt.float32
    BF = mybir.dt.bfloat16
    MM_DT = mybir.dt.float8e4 if use_fp8 else BF
    OUT_DT = mybir.dt.float8e4 if out_fp8 else BF
    nkt = KP if use_fp8 else KT
    KD = 2 if use_fp8 else 1  # DoubleRow sub-row dim

    nc = bacc.Bacc("TRN2", target_bir_lowering=False, debug=False)

    # chunked layouts: [P, chunk, k, kd, w] so any chunk range is
    # per-partition contiguous in DRAM (big DMA descriptors)
    ust_d = nc.dram_tensor("ust", [P, IT, nkt, KD, P], MM_DT, kind="ExternalInput").ap()
    vst_d = nc.dram_tensor("vst", [P, NCH, nkt, KD, JBLK], MM_DT, kind="ExternalInput").ap()
    v2b_d = nc.dram_tensor("v2b", [P, M], BF, kind="ExternalInput").ap()
    ubias_d = nc.dram_tensor("ubias", [P, IT], F32, kind="ExternalInput").ap()
    kout_d = nc.dram_tensor("kout", [NLOC, M], OUT_DT, kind="ExternalOutput").ap()

    pm = mybir.MatmulPerfMode.DoubleRow if use_fp8 else None

    with tile.TileContext(nc) as tc:
        with (
            tc.tile_pool(name="const", bufs=1) as const,
            tc.tile_pool(name="psum", bufs=2, space=bass.MemorySpace.PSUM) as psum,
            tc.tile_pool(name="e1p", bufs=4) as e1p,
            tc.tile_pool(name="outp", bufs=4) as outp,
        ):
            ubias_t = const.tile([P, IT], F32, tag="ubias")
            ust_t = const.tile([P, IT, nkt, KD, P], MM_DT, tag="ust")
            vst_t = const.tile([P, NCH, nkt, KD, JBLK], MM_DT, tag="vst")
            v2b_t = [const.tile([P, JG], BF, name=f"v2b{g}", tag=f"v2b{g}")
                     for g in range(NJG)]

            # DMA issue in consumption order, critical prefix split across
            # the two HWDGE rings (sync + scalar)
            nc.scalar.dma_start(ubias_t[:], ubias_d[:])
            nc.scalar.dma_start(ust_t[:, 0], ust_d[:, 0])
            nc.sync.dma_start(vst_t[:, 0], vst_d[:, 0])
            nc.sync.dma_start(v2b_t[0][:, 0:JBLK], v2b_d[:, 0:JBLK])
            nc.sync.dma_start(vst_t[:, 1:NJB], vst_d[:, 1:NJB])
            nc.scalar.dma_start(v2b_t[0][:, JBLK:JG], v2b_d[:, JBLK:JG])
            nc.scalar.dma_start(ust_t[:, 1:IT], ust_d[:, 1:IT])
            for g in range(1, NJG):
                nc.sync.dma_start(vst_t[:, g * NJB:(g + 1) * NJB],
                                  vst_d[:, g * NJB:(g + 1) * NJB])
                nc.sync.dma_start(v2b_t[g][:], v2b_d[:, g * JG:(g + 1) * JG])

            def mm(acc, it, k, jb, g, start, stop):
                ch = g * NJB + jb
                lhsT = (ust_t[:, it, k, :, :] if use_fp8
                        else ust_t[:, it, k, 0, :])
                rhs = (vst_t[:, ch, k, :, :] if use_fp8
                       else vst_t[:, ch, k, 0, :])
                nc.tensor.matmul(
                    acc[:, jb * JBLK:(jb + 1) * JBLK],
                    lhsT, rhs, start=start, stop=stop, perf_mode=pm,
                )

            def do_group(it, g, acc):
                for k in range(nkt):
                    for jb in range(NJB):
                        mm(acc, it, k, jb, g, k == 0, k == nkt - 1)

            def drain(it, g, acc, q0, q1, w):
                # elementwise chains over [q0*JBLK, q1*JBLK) in widths of w
                for j in range(q0 * JBLK, q1 * JBLK, w):
                    qs = slice(j, j + w)
                    e1 = e1p.tile([P, w], F32, tag="e1", name="e1")
                    nc.vector.tensor_add(e1[:], acc[:, qs], v2b_t[g][:, qs])
                    ot = outp.tile([P, w], OUT_DT, tag="ot", name="ot")
                    nc.scalar.activation(
                        ot[:], e1[:],
                        mybir.ActivationFunctionType.Exp,
                        bias=ubias_t[:, it:it + 1], scale=1.0,
                    )
                    nc.gpsimd.dma_start(
                        kout_d[it * P:(it + 1) * P, g * JG + j:g * JG + j + w],
                        ot[:],
                    )

            # ---- ramp: (g0, it0) as two 1024-wide half-groups with early
            # per-bank k-chain completion and 512-wide drains
            acc0 = psum.tile([P, JG], F32, tag="acc")
            for half in range(2):
                for jb in (2 * half, 2 * half + 1):
                    for k in range(nkt):
                        mm(acc0, 0, k, jb, 0, k == 0, k == nkt - 1)
                    drain(0, 0, acc0, jb, jb + 1, JBLK)

            # ---- steady state (g-major), narrow drains on ramp/tail edges
            for g in range(NJG):
                for it in range(IT):
                    if g == 0 and it == 0:
                        continue
                    acc = psum.tile([P, JG], F32, tag="acc")
                    do_group(it, g, acc)
                    if g == 0 and it == 1:
                        drain(it, g, acc, 0, NJB, 2 * JBLK)
                    elif g == NJG - 1 and it == IT - 1:
                        drain(it, g, acc, 0, NJB, JBLK)
                    else:
                        drain(it, g, acc, 0, NJB, JG)

    nc.compile()
    return nc


def _prep(U, V, weights, sn):
    s = np.exp(-weights[:, 0].astype(np.float64))
    Us = U.astype(np.float64) * s[None, :]
    Vs = V.astype(np.float64) * s[None, :]
    amax = max(np.abs(Us).max(), np.abs(Vs).max())
    use_fp8 = bool(amax < FP8_MAX)
    mmdt = F8 if use_fp8 else BF16

    # quantize, then compute row norms from the quantized values so the GEMM
    # identity sq = u2 + v2 - 2*cross holds for the on-device numbers
    Usq = Us.astype(mmdt)
    Vsq = Vs.astype(mmdt)
    u2 = np.sum(Usq.astype(np.float64) ** 2, axis=1)
    v2 = np.sum(Vsq.astype(np.float64) ** 2, axis=1)

    ust = np.ascontiguousarray(Usq.T)                    # [D, N]
    vst = np.ascontiguousarray(Vsq.T)                    # [D, M]
    nkt = KP if use_fp8 else KT
    KD = 2 if use_fp8 else 1
    # row d = (nkt-index*KD + kd)*128 + p; chunked [P, chunk, nkt, KD, w]
    ust = ust.reshape(nkt, KD, P, N // P, P).transpose(2, 3, 0, 1, 4)
    ust = np.ascontiguousarray(ust)
    vst = vst.reshape(nkt, KD, P, NCH, JBLK).transpose(2, 3, 0, 1, 4)
    vst = np.ascontiguousarray(vst)

    v2b = np.broadcast_to((-0.5 * v2).astype(BF16)[None, :], (P, M)).copy()
    bias_full = (2.0 * float(sn) - 0.5 * u2).astype(np.float32)  # [N]

    # fp8 output is used only when a sampled upper bound on the exponent
    # E = 2sn - 0.5*sq shows every output underflows fp32 to exactly 0.0
    # (fp8 and bf16 then store identical, exact zeros). Otherwise bf16.
    idx_i = np.arange(0, N, N // 1024)
    idx_j = np.arange(0, M, M // 1024)
    cross_s = Usq[idx_i].astype(np.float32) @ Vsq[idx_j].astype(np.float32).T
    E_s = (2.0 * float(sn) - 0.5 * u2[idx_i, None] - 0.5 * v2[None, idx_j]
           + cross_s)
    out_fp8 = bool(E_s.max() < -300.0)
    in_maps = []
    for c in range(NCORES):
        r0, r1 = c * NLOC, (c + 1) * NLOC
        ub = np.ascontiguousarray(
            bias_full[r0:r1].reshape(IT, P).T.astype(np.float32))
        # ust columns for this core: global i in [r0, r1) -> chunk dim
        uc = ust[:, r0 // P:r1 // P]
        in_maps.append({
            "ust": np.ascontiguousarray(uc),
            "vst": vst,
            "v2b": v2b,
            "ubias": ub,
        })
    return in_maps, use_fp8, out_fp8


def _run(inputs, trace=False, trace_kwargs=None):
    from concourse import bass_utils

    in_maps, use_fp8, out_fp8 = _prep(
        np.asarray(inputs["U"]), np.asarray(inputs["V"]),
        np.asarray(inputs["weights"]), np.asarray(inputs["sn"]),
    )
    key = ("fp8" if use_fp8 else "bf16") + ("_o8" if out_fp8 else "_o16")
    if key not in _cache:
        _cache[key] = _build(use_fp8, out_fp8)
    nc = _cache[key]
    res = bass_utils.run_bass_kernel_spmd(
        nc, in_maps, core_ids=list(range(NCORES)),
        trace=trace, **(trace_kwargs or {}),
    )
    out = np.empty((N, M), dtype=np.float32)
    for c in range(NCORES):
        out[c * NLOC:(c + 1) * NLOC, :] = res.results[c]["kout"].astype(np.float32)
    return out, res


def kernel(U, V, weights, sn):
    out, _ = _run({"U": U, "V": V, "weights": weights, "sn": sn})
    return out


# revision 7
# speedup vs baseline: 1.0706x; 1.0706x over previous
"""RBF/ARD covariance kernel K = exp(2*sn - 0.5 * ||s*(u_i - v_j)||^2) on 8 trn2 cores.

Strategy (sharding_hint): shard U rows across the 8 cores (each computes a
[1024, 8192] strip of K); V / weights / sn replicated.

Math: K = exp(E), E = 2*sn - 0.5*u2_i - 0.5*v2_j + (Us @ Vs.T)_ij with
Us = U*s, Vs = V*s, s = exp(-weights[:,0]), u2/v2 squared row norms of the
QUANTIZED Us/Vs (so E <= 2*sn up to fp32 accumulation noise and the
reference's max(sq,0) clamp cannot produce a visible difference).

Per core: fp8e4 GEMM with DoubleRow (contraction 512 = 2 passes of 2x128)
accumulated in fp32 PSUM; DVE adds the -0.5*v2_j broadcast row; ACT applies
exp(x + (2*sn - 0.5*u2_i)) via per-partition bias; bf16 store, host casts to
fp32. Falls back to bf16 GEMM if the scaled inputs exceed fp8e4 range.

Scheduling notes (from trace analysis):
- each dma_start costs ~650ns DGE issue time on its queue, and small strided
  descriptors kill early DMA bandwidth, so DRAM layouts are chunked to be
  per-partition contiguous per load ([P, chunk, k, 2, 512] for V);
- critical-path inputs are split over both HWDGE rings (sync + scalar);
- output DMAs go on the (otherwise idle) GpSimd SWDGE queue;
- DVE is the steady-state pacer (2222ns per [128,2048] group); ramp and
  tail use narrow chains so the elementwise pipeline starts/ends fast.
"""

import numpy as np
import ml_dtypes

N, M, D = 8192, 8192, 512
NCORES = 8
NLOC = N // NCORES          # 1024 U-rows per core
P = 128                     # partitions
KT = D // P                 # 4 contraction tiles of 128
KP = KT // 2                # 2 DoubleRow passes (2 k-tiles each)
IT = NLOC // P              # 8 i-tiles per core
JBLK = 512                  # matmul free dim (one PSUM bank fp32)
JG = 2048                   # j-group width (4 banks) for DVE/ACT/DMA batching
NJG = M // JG               # 4 j-groups
NJB = JG // JBLK            # 4 matmul j-blocks per group
NCH = M // JBLK             # 16 vst chunks of 512

F8 = ml_dtypes.float8_e4m3  # TRN float8e4 (max normal 240)
BF16 = ml_dtypes.bfloat16
FP8_MAX = 200.0             # safety margin under 240

_cache = {}


def _build(use_fp8, out_fp8):
    import concourse.bass as bass
    import concourse.mybir as mybir
    import concourse.tile as tile
    from concourse import bacc

    F32 = mybir.dt.float32
    BF = mybir.dt.bfloat16
    MM_DT = mybir.dt.float8e4 if use_fp8 else BF
    OUT_DT = mybir.dt.float8e4 if out_fp8 else BF
    nkt = KP if use_fp8 else KT
    KD = 2 if use_fp8 else 1  # DoubleRow sub-row dim

    nc = bacc.Bacc("TRN2", target_bir_lowering=False, debug=False)

    # chunked layouts: [P, chunk, k, kd, w] so any chunk range is
    # per-partition contiguous in DRAM (big DMA descriptors)
    ust_d = nc.dram_tensor("ust", [P, IT, nkt, KD, P], MM_DT, kind="ExternalInput").ap()
    vst_d = nc.dram_tensor("vst", [P, NCH, nkt, KD, JBLK], MM_DT, kind="ExternalInput").ap()
    v2b_d = nc.dram_tensor("v2b", [P, M], BF, kind="ExternalInput").ap()
    ubias_d = nc.dram_tensor("ubias", [P, IT], F32, kind="ExternalInput").ap()
    kout_d = nc.dram_tensor("kout", [NLOC, M], OUT_DT, kind="ExternalOutput").ap()

    pm = mybir.MatmulPerfMode.DoubleRow if use_fp8 else None

    with tile.TileContext(nc) as tc:
        with (
            tc.tile_pool(name="const", bufs=1) as const,
            tc.tile_pool(name="psum", bufs=2, space=bass.MemorySpace.PSUM) as psum,
            tc.tile_pool(name="e1p", bufs=4) as e1p,
            tc.tile_pool(name="outp", bufs=4) as outp,
        ):
            ubias_t = const.tile([P, IT], F32, tag="ubias")
            ust_t = const.tile([P, IT, nkt, KD, P], MM_DT, tag="ust")
            vst_t = const.tile([P, NCH, nkt, KD, JBLK], MM_DT, tag="vst")
            v2b_t = [const.tile([P, JG], BF, name=f"v2b{g}", tag=f"v2b{g}")
                     for g in range(NJG)]

            # DMA issue on one ring, in exact consumption order: completion
            # semaphores fire roughly in global byte order, so the critical
            # prefix (ust it0, vst chunk0, v2b piece0, ubias) goes first and
            # everything else streams behind it.
            nc.sync.dma_start(ust_t[:, 0], ust_d[:, 0])
            nc.sync.dma_start(vst_t[:, 0], vst_d[:, 0])
            nc.sync.dma_start(v2b_t[0][:, 0:JBLK], v2b_d[:, 0:JBLK])
            nc.sync.dma_start(ubias_t[:], ubias_d[:])
            nc.sync.dma_start(vst_t[:, 1], vst_d[:, 1])
            nc.sync.dma_start(v2b_t[0][:, JBLK:2 * JBLK],
                              v2b_d[:, JBLK:2 * JBLK])
            nc.sync.dma_start(vst_t[:, 2:NJB], vst_d[:, 2:NJB])
            nc.sync.dma_start(v2b_t[0][:, 2 * JBLK:JG], v2b_d[:, 2 * JBLK:JG])
            nc.sync.dma_start(ust_t[:, 1:IT], ust_d[:, 1:IT])
            for g in range(1, NJG):
                nc.sync.dma_start(vst_t[:, g * NJB:(g + 1) * NJB],
                                  vst_d[:, g * NJB:(g + 1) * NJB])
                nc.sync.dma_start(v2b_t[g][:], v2b_d[:, g * JG:(g + 1) * JG])

            def mm(acc, it, k, jb, g, start, stop):
                ch = g * NJB + jb
                lhsT = (ust_t[:, it, k, :, :] if use_fp8
                        else ust_t[:, it, k, 0, :])
                rhs = (vst_t[:, ch, k, :, :] if use_fp8
                       else vst_t[:, ch, k, 0, :])
                nc.tensor.matmul(
                    acc[:, jb * JBLK:(jb + 1) * JBLK],
                    lhsT, rhs, start=start, stop=stop, perf_mode=pm,
                )

            def do_group(it, g, acc):
                for k in range(nkt):
                    for jb in range(NJB):
                        mm(acc, it, k, jb, g, k == 0, k == nkt - 1)

            def drain(it, g, acc, q0, q1, w):
                # elementwise chains over [q0*JBLK, q1*JBLK) in widths of w
                for j in range(q0 * JBLK, q1 * JBLK, w):
                    qs = slice(j, j + w)
                    e1 = e1p.tile([P, w], F32, tag="e1", name="e1")
                    nc.vector.tensor_add(e1[:], acc[:, qs], v2b_t[g][:, qs])
                    ot = outp.tile([P, w], OUT_DT, tag="ot", name="ot")
                    nc.scalar.activation(
                        ot[:], e1[:],
                        mybir.ActivationFunctionType.Exp,
                        bias=ubias_t[:, it:it + 1], scale=1.0,
                    )
                    nc.gpsimd.dma_start(
                        kout_d[it * P:(it + 1) * P, g * JG + j:g * JG + j + w],
                        ot[:],
                    )

            # ---- ramp: (g0, it0) as two 1024-wide half-groups with early
            # per-bank k-chain completion and 512-wide drains
            acc0 = psum.tile([P, JG], F32, tag="acc")
            for half in range(2):
                for jb in (2 * half, 2 * half + 1):
                    for k in range(nkt):
                        mm(acc0, 0, k, jb, 0, k == 0, k == nkt - 1)
                    drain(0, 0, acc0, jb, jb + 1, JBLK)

            # ---- steady state (g-major), narrow drains on ramp/tail edges
            for g in range(NJG):
                for it in range(IT):
                    if g == 0 and it == 0:
                        continue
                    acc = psum.tile([P, JG], F32, tag="acc")
                    do_group(it, g, acc)
                    if g == 0 and it == 1:
                        drain(it, g, acc, 0, NJB, 2 * JBLK)
                    elif g == NJG - 1 and it == IT - 1:
                        drain(it, g, acc, 0, NJB, JBLK)
                    else:
                        drain(it, g, acc, 0, NJB, JG)

    nc.compile()
    return nc


def _prep(U, V, weights, sn):
    s = np.exp(-weights[:, 0].astype(np.float64))
    Us = U.astype(np.float64) * s[None, :]
    Vs = V.astype(np.float64) * s[None, :]
    amax = max(np.abs(Us).max(), np.abs(Vs).max())
    use_fp8 = bool(amax < FP8_MAX)
    mmdt = F8 if use_fp8 else BF16

    # quantize, then compute row norms from the quantized values so the GEMM
    # identity sq = u2 + v2 - 2*cross holds for the on-device numbers
    Usq = Us.astype(mmdt)
    Vsq = Vs.astype(mmdt)
    u2 = np.sum(Usq.astype(np.float64) ** 2, axis=1)
    v2 = np.sum(Vsq.astype(np.float64) ** 2, axis=1)

    ust = np.ascontiguousarray(Usq.T)                    # [D, N]
    vst = np.ascontiguousarray(Vsq.T)                    # [D, M]
    nkt = KP if use_fp8 else KT
    KD = 2 if use_fp8 else 1
    # row d = (nkt-index*KD + kd)*128 + p; chunked [P, chunk, nkt, KD, w]
    ust = ust.reshape(nkt, KD, P, N // P, P).transpose(2, 3, 0, 1, 4)
    ust = np.ascontiguousarray(ust)
    vst = vst.reshape(nkt, KD, P, NCH, JBLK).transpose(2, 3, 0, 1, 4)
    vst = np.ascontiguousarray(vst)

    v2b = np.broadcast_to((-0.5 * v2).astype(BF16)[None, :], (P, M)).copy()
    bias_full = (2.0 * float(sn) - 0.5 * u2).astype(np.float32)  # [N]

    # fp8 output is used only when a sampled upper bound on the exponent
    # E = 2sn - 0.5*sq shows every output underflows fp32 to exactly 0.0
    # (fp8 and bf16 then store identical, exact zeros). Otherwise bf16.
    idx_i = np.arange(0, N, N // 1024)
    idx_j = np.arange(0, M, M // 1024)
    cross_s = Usq[idx_i].astype(np.float32) @ Vsq[idx_j].astype(np.float32).T
    E_s = (2.0 * float(sn) - 0.5 * u2[idx_i, None] - 0.5 * v2[None, idx_j]
           + cross_s)
    out_fp8 = bool(E_s.max() < -300.0)
    in_maps = []
    for c in range(NCORES):
        r0, r1 = c * NLOC, (c + 1) * NLOC
        ub = np.ascontiguousarray(
            bias_full[r0:r1].reshape(IT, P).T.astype(np.float32))
        # ust columns for this core: global i in [r0, r1) -> chunk dim
        uc = ust[:, r0 // P:r1 // P]
        in_maps.append({
            "ust": np.ascontiguousarray(uc),
            "vst": vst,
            "v2b": v2b,
            "ubias": ub,
        })
    return in_maps, use_fp8, out_fp8


def _run(inputs, trace=False, trace_kwargs=None):
    from concourse import bass_utils

    in_maps, use_fp8, out_fp8 = _prep(
        np.asarray(inputs["U"]), np.asarray(inputs["V"]),
        np.asarray(inputs["weights"]), np.asarray(inputs["sn"]),
    )
    key = ("fp8" if use_fp8 else "bf16") + ("_o8" if out_fp8 else "_o16")
    if key not in _cache:
        _cache[key] = _build(use_fp8, out_fp8)
    nc = _cache[key]
    res = bass_utils.run_bass_kernel_spmd(
        nc, in_maps, core_ids=list(range(NCORES)),
        trace=trace, **(trace_kwargs or {}),
    )
    out = np.empty((N, M), dtype=np.float32)
    for c in range(NCORES):
        out[c * NLOC:(c + 1) * NLOC, :] = res.results[c]["kout"].astype(np.float32)
    return out, res


def kernel(U, V, weights, sn):
    out, _ = _run({"U": U, "V": V, "weights": weights, "sn": sn})
    return out
